# revision 12
# baseline (speedup 1.0000x reference)
"""Trainium2 Bass kernel for nn_Attention_3135326126702.

Computation (see reference): ViT-style attention block on x:(2,384,56,56).
  q/k/v/proj are 1x1 conv + eval-mode BN (affine, folded into weights on host).
  8 heads, key_dim=16, d=64, N=3136 positions, softmax(q@k) (no scale) @ v,
  relu, proj conv+BN.

Sharding: 8 cores = (batch b in {0,1}) x (query-quarter qq in {0..3}).
Each core computes the FULL K/V for its batch (duplicated across the 4
cores of a batch -- cheap) and attention + proj for its 784 query columns.
No collectives. Host rolls x columns per-core so the query slice is always
columns 0:784 (key order permutation is softmax-invariant).

On-chip layout is fully transposed (channels/keys on partitions) so no
transposes are ever needed:
  scoresT[key, q] = matmul(lhsT=K_blk[:, key_tile], rhs=Q[:, q])
    K_blk is BLOCK-DIAGONAL over heads: per head a [128, N] fp16 matrix
    whose rows 32*hp..32*hp+16 hold that head's K channels, zeros
    elsewhere; Q is stored with all 4 heads of a tile stacked the same
    way. This makes every scores matmul a full K=128 contraction, which
    keeps the PE HAM clock-gate at 8/8 (2.4 GHz) -- measured: K<128
    matmuls don't register as PE activity and the clock halves.
  exp on ScalarE (no max subtraction: scores are O(+-20) here; fp32 exp
    overflows only at 88).
  xxT_aug[65, q] += matmul(lhsT=VT_aug[key_tile, 65] bf16, rhs=exp_scoresT bf16)
    VT_aug col 64 == 1.0 -> row 64 accumulates the softmax denominator.
    (bf16 because exp values reach ~1e7: too big for fp16.)
  normalize: broadcast denom row via 1-row f32r matmul, fast-approx
    reciprocal, then xx = max(xxT,0) * recip in one DVE op.
  proj: accumulate 8 per-head K=64 fp16 matmuls, + bias, DMA out.

dtypes: convs in float32r (full-rate fp32 with ~1e-4 rounding; regular
fp32 matmul runs at 1/4 rate), scores/proj fp16, attnV bf16.
"""

import numpy as np

import concourse.bass as bass
import concourse.mybir as mybir
from concourse import bacc
import concourse.tile as tile
from concourse.bass_utils import run_bass_kernel_spmd

F32 = mybir.dt.float32
F32R = mybir.dt.float32r
BF16 = mybir.dt.bfloat16
FP16 = mybir.dt.float16
AF = mybir.ActivationFunctionType
ALU = mybir.AluOpType

EPS = 1e-5
B = 2
CIN = 384          # input channels
N = 3136           # 56*56 positions
NQ = 784           # query positions per core (N/4)
NH = 8             # heads
KD = 16            # key dim per head
NKT = 25           # key tiles of 128 (last has 64)
FCH = ((0, 512), (512, 272))  # 784-wide free dim split at the PSUM bank edge
N_CORES = 8

_CACHE = {}


def _key_tiles():
    for kt in range(NKT):
        p0 = kt * 128
        yield kt, p0, min(128, N - p0)


def build_nc() -> bass.Bass:
    nc = bacc.Bacc()

    xb = nc.declare_dram_parameter("xb", [CIN, N], FP16, isOutput=False)[:]
    wqT = nc.declare_dram_parameter("wqT", [CIN, 256], FP16, isOutput=False)[:]
    wkT = nc.declare_dram_parameter("wkT", [CIN, 256], FP16, isOutput=False)[:]
    wvT = nc.declare_dram_parameter("wvT", [CIN, 512], FP16, isOutput=False)[:]
    wpT = nc.declare_dram_parameter("wpT", [64, NH, CIN], FP16, isOutput=False)[:]
    bq = nc.declare_dram_parameter("bq", [128, 2], F32, isOutput=False)[:]
    bv = nc.declare_dram_parameter("bv", [1, 512], FP16, isOutput=False)[:]
    bp = nc.declare_dram_parameter("bp", [128, 3], F32, isOutput=False)[:]
    ones_in = nc.declare_dram_parameter("ones_in", [1, 128], FP16, isOutput=False)[:]
    out = nc.declare_dram_parameter("out", [CIN, NQ], F32, isOutput=True)[:]

    with tile.TileContext(nc) as tc:
        with (
            tc.tile_pool(name="const", bufs=1) as cst,
            tc.tile_pool(name="work", bufs=3) as wk,
            tc.tile_pool(name="ps", bufs=2, space="PSUM") as ps,
            tc.tile_pool(name="dram", bufs=2, space="DRAM") as dr,
        ):
            # ---- constants / weights ----
            wq_sb = cst.tile([128, 3, 256], FP16)
            nc.sync.dma_start(out=wq_sb, in_=wqT.rearrange("(o p) m -> p o m", p=128))
            wk_sb = cst.tile([128, 3, 256], FP16)
            nc.sync.dma_start(out=wk_sb, in_=wkT.rearrange("(o p) m -> p o m", p=128))
            wv_sb = cst.tile([128, 3, 512], FP16)
            nc.sync.dma_start(out=wv_sb, in_=wvT.rearrange("(o p) m -> p o m", p=128))
            wp_sb = cst.tile([64, NH, CIN], FP16)
            nc.sync.dma_start(out=wp_sb, in_=wpT)
            bq_sb = cst.tile([128, 2], F32)
            nc.sync.dma_start(out=bq_sb, in_=bq)
            bv_sb = cst.tile([1, 512], FP16)
            nc.sync.dma_start(out=bv_sb, in_=bv)
            bp_sb = cst.tile([128, 3], F32)
            nc.sync.dma_start(out=bp_sb, in_=bp)

            ones_sb = cst.tile([1, 128], FP16)
            nc.sync.dma_start(out=ones_sb, in_=ones_in)

            # ---- persistent activations ----
            q_sb = cst.tile([128, 2, NQ], FP16)      # head h at rows 32(h%4)+0:16, tile h//4
            kblk_sb = cst.tile([128, NH, N], FP16)   # block-diagonal K per head
            vvT_sb = cst.tile([128, NKT, NH, 65], BF16)  # [key, kt, head, 64 V + ones]
            xx_sb = cst.tile([64, NH, NQ], FP16)     # normalized+relu'd per-head xx
            out_sb = cst.tile([128, 3, NQ], F32)

            nc.gpsimd.memset(kblk_sb, 0.0)
            nc.gpsimd.memset(vvT_sb[:, :, :, 64:65], 1.0)

            x_sb = cst.tile([128, 3, N], FP16)
            x_src = xb.rearrange("(o p) n -> p o n", p=128)
            for c in range(3):
                nc.sync.dma_start(out=x_sb[:, c, :], in_=x_src[:, c, :])

            ppart_sb = cst.tile([128, 3, NQ], F32)   # proj partial (heads 0-3)

            # ---- emission order == scheduler priority ----
            # The kernel is ScalarE(exp)-bound, so start head 0's attention as
            # early as deps allow and spread the remaining conv/proj work into
            # the PE slack of the head loop.

            def q_conv(t):
                qp = ps.tile([128, NQ], F32, tag="sc", bufs=2, name="qp")
                for f0, fl in FCH:
                    for c in range(3):
                        nc.tensor.matmul(
                            qp[:, f0:f0 + fl],
                            lhsT=wq_sb[:, c, 128 * t:128 * t + 128],
                            rhs=x_sb[:, c, f0:f0 + fl],
                            start=(c == 0), stop=(c == 2),
                        )
                nc.scalar.activation(
                    out=q_sb[:, t, :], in_=qp, func=AF.Identity,
                    bias=bq_sb[:, t:t + 1],
                )

            def k_conv(t, fi):
                # K conv fchunk -> scatter rows into the block-diagonal K
                # (BN bias dropped: per-query-constant shift is softmax-invariant)
                g0 = fi * NQ
                kp = ps.tile([128, NQ], F32, tag="sc", bufs=2, name="kp")
                for f0, fl in FCH:
                    for c in range(3):
                        nc.tensor.matmul(
                            kp[:, f0:f0 + fl],
                            lhsT=wk_sb[:, c, 128 * t:128 * t + 128],
                            rhs=x_sb[:, c, g0 + f0:g0 + f0 + fl],
                            start=(c == 0), stop=(c == 2),
                        )
                for hp in range(4):
                    nc.vector.tensor_copy(
                        out=kblk_sb[32 * hp:32 * hp + KD, 4 * t + hp, g0:g0 + NQ],
                        in_=kp[32 * hp:32 * hp + KD, :],
                    )

            def vt_conv(kt, p0, pl):
                # VT conv: out[key_tile, 512] = x_chunk.T @ wv.T (+ bias row)
                vp = ps.tile([128, NQ], F32, tag="sc", bufs=2, name="vp")
                nc.tensor.matmul(
                    vp[:pl, 0:512], lhsT=ones_sb[:, :pl], rhs=bv_sb,
                    start=True, stop=False,
                )
                for c in range(3):
                    nc.tensor.matmul(
                        vp[:pl, 0:512],
                        lhsT=x_sb[:, c, p0:p0 + pl],
                        rhs=wv_sb[:, c, :],
                        start=False, stop=(c == 2),
                    )
                nc.scalar.activation(
                    out=vvT_sb[:pl, kt, :, 0:64],
                    in_=vp[:pl, 0:512].rearrange("p (h d) -> p h d", h=NH),
                    func=AF.Copy,
                )

            def head(h, side_work):
                ht = h // 4
                xxp = ps.tile([65, NQ], F32, tag="xx", bufs=2, name="xxp")
                for kt, p0, pl in _key_tiles():
                    if kt in side_work:
                        side_work[kt]()
                    sp = ps.tile([128, NQ], F32, tag="sc", bufs=2, name="sp")
                    for f0, fl in FCH:
                        nc.tensor.matmul(
                            sp[:pl, f0:f0 + fl],
                            lhsT=kblk_sb[:, h, p0:p0 + pl],
                            rhs=q_sb[:, ht, f0:f0 + fl],
                            start=True, stop=True,
                        )
                    es = wk.tile([128, NQ], BF16, tag="es", bufs=4, name="es")
                    nc.scalar.activation(out=es[:pl], in_=sp[:pl], func=AF.Exp)
                    for f0, fl in FCH:
                        nc.tensor.matmul(
                            xxp[:, f0:f0 + fl],
                            lhsT=vvT_sb[:pl, kt, h, :],
                            rhs=es[:pl, f0:f0 + fl],
                            start=(kt == 0), stop=(kt == NKT - 1),
                        )
                # normalize + relu: xx = max(xxT,0)/denom; denom = row 64.
                # Broadcast the denom row across partitions via a DRAM-bounce
                # DMA (keeps the PE stream dense at head boundaries).
                dn = wk.tile([65, NQ], F32, tag="dn", bufs=2, name="dn")
                nc.vector.tensor_copy(out=dn[64:65, :], in_=xxp[64:65, :])
                dnd = dr.tile([1, NQ], F32, name="dnd")
                nc.sync.dma_start(out=dnd, in_=dn[64:65, :])
                dnb = wk.tile([64, NQ], F32, tag="dnb", bufs=2, name="dnb")
                nc.gpsimd.dma_start(
                    out=dnb, in_=dnd[0, :].partition_broadcast(64)
                )
                rec = wk.tile([64, NQ], F32, tag="rec", bufs=2, name="rec")
                with nc.allow_low_precision(reason="~18-bit recip for softmax denom"):
                    nc.vector.reciprocal_approx_fast(out=rec, in_=dnb)
                nc.vector.scalar_tensor_tensor(
                    out=xx_sb[:, h, :], in0=xxp[0:64, :], scalar=0.0, in1=rec,
                    op0=ALU.max, op1=ALU.mult,
                )

            def proj_pass_a():
                # accumulate heads 0-3, park in SBUF
                for t in range(3):
                    pp = ps.tile([128, NQ], F32, tag="sc", bufs=2, name="pp")
                    for f0, fl in FCH:
                        for h in range(4):
                            nc.tensor.matmul(
                                pp[:, f0:f0 + fl],
                                lhsT=wp_sb[:, h, 128 * t:128 * t + 128],
                                rhs=xx_sb[:, h, f0:f0 + fl],
                                start=(h == 0), stop=(h == 3),
                            )
                    nc.vector.tensor_copy(out=ppart_sb[:, t, :], in_=pp)

            def proj_pass_b():
                for t in range(3):
                    pp = ps.tile([128, NQ], F32, tag="sc", bufs=2, name="pp")
                    for f0, fl in FCH:
                        for h in range(4, 8):
                            nc.tensor.matmul(
                                pp[:, f0:f0 + fl],
                                lhsT=wp_sb[:, h, 128 * t:128 * t + 128],
                                rhs=xx_sb[:, h, f0:f0 + fl],
                                start=(h == 4), stop=(h == 7),
                            )
                    nc.vector.scalar_tensor_tensor(
                        out=out_sb[:, t, :], in0=pp, scalar=bp_sb[:, t:t + 1],
                        in1=ppart_sb[:, t, :], op0=ALU.add, op1=ALU.add,
                    )
                nc.sync.dma_start(
                    out=out.rearrange("(o p) n -> p o n", p=128), in_=out_sb
                )

            q_conv(0)
            for fi in range(4):
                k_conv(0, fi)
            # head 0 races the VT conv tile-by-tile
            head(0, {kt: (lambda kt=kt, p0=p0, pl=pl: vt_conv(kt, p0, pl))
                     for kt, p0, pl in _key_tiles()})
            head(1, {1: lambda: q_conv(1),
                     8: lambda: k_conv(1, 0), 16: lambda: k_conv(1, 1)})
            head(2, {1: lambda: k_conv(1, 2), 12: lambda: k_conv(1, 3)})
            head(3, {})
            head(4, {1: proj_pass_a})
            head(5, {})
            head(6, {})
            head(7, {})
            proj_pass_b()

    nc.compile()
    return nc


def _fold_bn(w, g, b, m, v):
    s = (g / np.sqrt(v + EPS)).astype(np.float32)
    return (s[:, None] * w).astype(np.float32), (b - m * s).astype(np.float32)


def _pad_heads(w, bias):
    """[128=8h*16kd, CIN] -> [256=8h*32, CIN] with rows 16..31 of each head zero."""
    wp = np.zeros((256, CIN), np.float32)
    bp_ = np.zeros((256,), np.float32)
    for h in range(NH):
        wp[32 * h:32 * h + KD] = w[KD * h:KD * h + KD]
        bp_[32 * h:32 * h + KD] = bias[KD * h:KD * h + KD]
    return wp, bp_


def make_in_maps(inputs):
    """Host-side prep: fold BN, reorder/pad weight rows, roll x per core."""
    wq, bq_ = _fold_bn(inputs["wq"], inputs["qg"], inputs["qb"], inputs["qm"], inputs["qv"])
    wkm, _ = _fold_bn(inputs["wk"], inputs["kg"], inputs["kb"], inputs["km"], inputs["kvv"])
    wv, bv_ = _fold_bn(inputs["wv"], inputs["vg"], inputs["vb"], inputs["vm"], inputs["vvv"])
    wp, bp_ = _fold_bn(inputs["wp"], inputs["pg"], inputs["pb"], inputs["pm"], inputs["pvv"])

    wq_p, bq_p = _pad_heads(wq, bq_)
    wk_p, _ = _pad_heads(wkm, np.zeros(128, np.float32))

    shared = {
        "wqT": np.ascontiguousarray(wq_p.T).astype(np.float16),
        "wkT": np.ascontiguousarray(wk_p.T).astype(np.float16),
        "wvT": np.ascontiguousarray(wv.T).astype(np.float16),
        "wpT": np.ascontiguousarray(
            wp.T.reshape(NH, 64, CIN).transpose(1, 0, 2)).astype(np.float16),
        "bq": np.ascontiguousarray(bq_p.reshape(2, 128).T),        # [128, 2]
        "bv": bv_.reshape(1, 512).astype(np.float16),
        "bp": np.ascontiguousarray(bp_.reshape(3, 128).T),         # [128, 3]
        "ones_in": np.ones((1, 128), np.float16),
    }
    x = np.asarray(inputs["x"], np.float32).reshape(B, CIN, N)
    in_maps = []
    for core in range(N_CORES):
        b, qq = divmod(core, 4)
        xr = np.ascontiguousarray(np.roll(x[b], -NQ * qq, axis=1)).astype(np.float16)
        in_maps.append({"xb": xr, **shared})
    return in_maps


def assemble(results):
    out = np.empty((B, CIN, N), np.float32)
    for core in range(N_CORES):
        b, qq = divmod(core, 4)
        out[b, :, NQ * qq:NQ * (qq + 1)] = results[core]["out"]
    return out.reshape(B, CIN, 56, 56)


def kernel(**inputs) -> np.ndarray:
    if "nc" not in _CACHE:
        _CACHE["nc"] = build_nc()
    nc = _CACHE["nc"]
    in_maps = make_in_maps(inputs)
    res = run_bass_kernel_spmd(nc, in_maps, core_ids=list(range(N_CORES)))
    return assemble(res.results)
